# revision 5
# baseline (speedup 1.0000x reference)
"""Self-contained Trainium2 Bass kernel for single-head attention.

Problem (per batch b of 8):
    q = Wq @ X[b] + bq            (dattn=1024, lx=2048)
    k = Wk @ Z[b] + bk            (dattn=1024, lz=2048)
    v = Wv @ Z[b] + bv            (dout=1024,  lz=2048)
    S = k^T q                     (lz, lx)
    attn = softmax(where(mask, S, -inf) / sqrt(dattn), axis=lz)
    out[b] = v @ attn             (dout, lx)

Strategy:
  * Pure data parallelism: core b computes batch b (8 batches / 8 cores, no
    collectives).
  * All matmuls run as float32r (TF32-like, ~13 mantissa bits) which is 4x
    faster than fp32 on the PE array for moving dims >= 256.  Every SBUF
    tile feeding a matmul is declared float32r and produced as such
    (DMA bitcast or engine output conversion) to satisfy walrus.
  * Softmax without max-subtraction (scores are O(1) after the 1/32 scale, so
    exp never overflows): E = exp((S + maskbias)/32) is produced directly in
    (z, x) layout, the denominator D[x] = sum_z E[z,x] comes from a matmul
    with a ones vector, and the output is built transposed,
    OT = E^T @ vT, so dividing by D is a per-partition scalar multiply.
    The host transposes OT back and adds bv (exact: attention columns sum
    to 1, so the bv contribution is just bv broadcast).
  * The boolean mask is classified on the host per (128-z-tile x 256-x-block)
    into skip / fully-unmasked / partial.  Skipped blocks generate no compute;
    partial blocks add a packed additive-bias tile (0 or -1e30).  This is
    fully general in the mask, and skips ~44% of attention work for the
    causal mask.
"""

import math
import os
import sys

import numpy as np

P = 128            # partitions
D = 1024           # dx = dz (contraction dim of the projections)
DA = 1024          # dattn
DO = 1024          # dout
LX = 2048
LZ = 2048
BS = 8
KT = D // P        # contraction tiles for projections (8)
MA = DA // P       # dattn tiles (8)
NZT = LZ // P      # z tiles (16)
BX = 256           # attention x-block
NXB = LX // BX     # 8
CH = 512           # projection-phase column chunk
NB = 512           # PSUM bank free-dim (fp32)
SCALE = 1.0 / math.sqrt(DA)
NEG = -1.0e30

_CACHE = {}


def _get_concourse():
    try:
        import concourse.bass  # noqa: F401
    except ImportError:
        for p in ("/opt/trn_rl_repo", "/root/.axon_site/_ro/trn_rl_repo"):
            if os.path.isdir(p) and p not in sys.path:
                sys.path.insert(0, p)
    import concourse.bass as bass
    import concourse.mybir as mybir
    import concourse.tile as tile
    from concourse import bacc, bass_utils

    return bass, mybir, tile, bacc, bass_utils


def _classify(mask):
    """Per (z-tile, x-block) mask status: 0 skip, 1 fully-unmasked, 2 partial."""
    status = np.zeros((NZT, NXB), dtype=np.int32)
    for zt in range(NZT):
        for i in range(NXB):
            sub = mask[zt * P:(zt + 1) * P, i * BX:(i + 1) * BX]
            if sub.all():
                status[zt, i] = 1
            elif sub.any():
                status[zt, i] = 2
    return status


def _build(status_key):
    bass, mybir, tile, bacc, bass_utils = _get_concourse()
    f32 = mybir.dt.float32
    f32r = mybir.dt.float32r
    AF = mybir.ActivationFunctionType
    ADD = mybir.AluOpType.add

    def r(ap):
        return ap.bitcast(f32r)

    status = np.array(status_key, dtype=np.int32).reshape(NZT, NXB)
    partial_pairs = [(zt, i) for i in range(NXB) for zt in range(NZT)
                     if status[zt, i] == 2]
    n_partial = max(1, len(partial_pairs))
    partial_idx = {pair: j for j, pair in enumerate(partial_pairs)}

    nc = bacc.Bacc("TRN2", target_bir_lowering=False, debug=False,
                   num_devices=1)
    Xd = nc.dram_tensor("X", (D, LX), f32, kind="ExternalInput").ap()
    Zd = nc.dram_tensor("Z", (D, LZ), f32, kind="ExternalInput").ap()
    MBd = nc.dram_tensor("MBP", (n_partial, P, BX), f32,
                         kind="ExternalInput").ap()
    WqTd = nc.dram_tensor("WqT", (D, DA), f32, kind="ExternalInput").ap()
    WkTd = nc.dram_tensor("WkT", (D, DA), f32, kind="ExternalInput").ap()
    WvTd = nc.dram_tensor("WvT", (D, DO), f32, kind="ExternalInput").ap()
    bqd = nc.dram_tensor("bq", (DA, 1), f32, kind="ExternalInput").ap()
    bkd = nc.dram_tensor("bk", (DA, 1), f32, kind="ExternalInput").ap()
    onesd = nc.dram_tensor("ones", (P, 2), f32, kind="ExternalInput").ap()
    qsd = nc.dram_tensor("qs", (DA, LX), f32, kind="Internal").ap()
    OTd = nc.dram_tensor("OT", (LX, DO), f32, kind="ExternalOutput").ap()

    xv = r(Xd.rearrange("(t p) l -> p t l", p=P))
    zv = r(Zd.rearrange("(t p) l -> p t l", p=P))
    wqv = r(WqTd.rearrange("(t p) d -> p t d", p=P))
    wkv = r(WkTd.rearrange("(t p) d -> p t d", p=P))
    wvv = r(WvTd.rearrange("(t p) d -> p t d", p=P))
    qsv = r(qsd.rearrange("(t p) l -> p t l", p=P))

    with tile.TileContext(nc) as tc:
        with tc.tile_pool(name="const", bufs=1) as cpool, \
             tc.tile_pool(name="kres", bufs=1) as kpool, \
             tc.tile_pool(name="vres", bufs=1) as vpool:
            bq_sb = cpool.tile([P, MA, 1], f32)
            nc.sync.dma_start(bq_sb, bqd.rearrange("(t p) o -> p t o", p=P))
            bk_sb = cpool.tile([P, MA, 1], f32)
            nc.sync.dma_start(bk_sb, bkd.rearrange("(t p) o -> p t o", p=P))
            ones_sb = cpool.tile([P, 2], f32r)
            nc.sync.dma_start(ones_sb, r(onesd))

            k_sb = kpool.tile([P, MA, LZ], f32r)      # k: (dattn, lz)
            vt_sb = vpool.tile([P, NZT, DO], f32r)    # v^T: (lz, dout)

            # ---- Phase V: vT = Z^T @ WvT ----
            with tc.tile_pool(name="wv", bufs=1) as wvp, \
                 tc.tile_pool(name="zin", bufs=2) as zinp, \
                 tc.tile_pool(name="psv", bufs=4, space="PSUM") as psp:
                wvt_sb = wvp.tile([P, KT, DO], f32r)
                nc.sync.dma_start(wvt_sb, wvv)
                for c in range(LZ // CH):
                    z_sb = zinp.tile([P, KT, CH], f32r)
                    nc.sync.dma_start(z_sb, zv[:, :, c * CH:(c + 1) * CH])
                    for m in range(CH // P):
                        for n in range(DO // NB):
                            vps = psp.tile([P, NB], f32)
                            for kt in range(KT):
                                nc.tensor.matmul(
                                    vps,
                                    z_sb[:, kt, m * P:(m + 1) * P],
                                    wvt_sb[:, kt, n * NB:(n + 1) * NB],
                                    start=(kt == 0), stop=(kt == KT - 1))
                            nc.vector.tensor_copy(
                                vt_sb[:, c * (CH // P) + m,
                                      n * NB:(n + 1) * NB], vps)

            # ---- Phase K: k = Wk @ Z + bk ----
            with tc.tile_pool(name="wk", bufs=1) as wkp, \
                 tc.tile_pool(name="zin2", bufs=2) as zinp, \
                 tc.tile_pool(name="psk", bufs=4, space="PSUM") as psp:
                wkt_sb = wkp.tile([P, KT, DA], f32r)
                nc.sync.dma_start(wkt_sb, wkv)
                for c in range(LZ // CH):
                    z_sb = zinp.tile([P, KT, CH], f32r)
                    nc.sync.dma_start(z_sb, zv[:, :, c * CH:(c + 1) * CH])
                    for m in range(MA):
                        kps = psp.tile([P, CH], f32)
                        for kt in range(KT):
                            nc.tensor.matmul(
                                kps,
                                wkt_sb[:, kt, m * P:(m + 1) * P],
                                z_sb[:, kt, :],
                                start=(kt == 0), stop=(kt == KT - 1))
                        nc.scalar.activation(
                            k_sb[:, m, c * CH:(c + 1) * CH], kps,
                            AF.Identity, bias=bk_sb[:, m, :], scale=1.0)

            # ---- Phase Q: q = Wq @ X + bq, spilled to DRAM ----
            with tc.tile_pool(name="wq", bufs=1) as wqp, \
                 tc.tile_pool(name="xin", bufs=2) as xinp, \
                 tc.tile_pool(name="qst", bufs=4) as qstp, \
                 tc.tile_pool(name="psq", bufs=4, space="PSUM") as psp:
                wqt_sb = wqp.tile([P, KT, DA], f32r)
                nc.sync.dma_start(wqt_sb, wqv)
                for c in range(LX // CH):
                    x_sb = xinp.tile([P, KT, CH], f32r)
                    nc.sync.dma_start(x_sb, xv[:, :, c * CH:(c + 1) * CH])
                    for m in range(MA):
                        qps = psp.tile([P, CH], f32)
                        for kt in range(KT):
                            nc.tensor.matmul(
                                qps,
                                wqt_sb[:, kt, m * P:(m + 1) * P],
                                x_sb[:, kt, :],
                                start=(kt == 0), stop=(kt == KT - 1))
                        qstage = qstp.tile([P, CH], f32r)
                        nc.scalar.activation(qstage, qps, AF.Identity,
                                             bias=bq_sb[:, m, :], scale=1.0)
                        nc.sync.dma_start(
                            qsv[:, m, c * CH:(c + 1) * CH], qstage)

            # ---- Attention ----
            with tc.tile_pool(name="qblk", bufs=2) as qblkp, \
                 tc.tile_pool(name="ebuf", bufs=2) as epool, \
                 tc.tile_pool(name="mbuf", bufs=4) as mpool, \
                 tc.tile_pool(name="otb", bufs=3) as otp, \
                 tc.tile_pool(name="drb", bufs=4) as drp, \
                 tc.tile_pool(name="pss", bufs=2, space="PSUM") as spsp, \
                 tc.tile_pool(name="pso", bufs=2, space="PSUM") as opsp, \
                 tc.tile_pool(name="psd", bufs=2, space="PSUM") as dpsp:
                for i in range(NXB):
                    active = [zt for zt in range(NZT) if status[zt, i] != 0]
                    q_sb = qblkp.tile([P, MA, BX], f32r)
                    nc.sync.dma_start(q_sb, qsv[:, :, i * BX:(i + 1) * BX])
                    e_sb = epool.tile([P, NZT, BX], f32r)
                    for zt in active:
                        sps = spsp.tile([P, BX], f32)
                        for kt in range(MA):
                            nc.tensor.matmul(
                                sps,
                                k_sb[:, kt, zt * P:(zt + 1) * P],
                                q_sb[:, kt, :],
                                start=(kt == 0), stop=(kt == MA - 1))
                        if status[zt, i] == 2:
                            mb_sb = mpool.tile([P, BX], f32)
                            nc.sync.dma_start(
                                mb_sb, MBd[partial_idx[(zt, i)]])
                            nc.vector.tensor_tensor(sps, sps, mb_sb, op=ADD)
                        nc.scalar.activation(e_sb[:, zt, :], sps, AF.Exp,
                                             scale=SCALE)
                    for ms in range(BX // P):
                        ot = otp.tile([P, DO], f32)
                        if active:
                            ops = opsp.tile([P, DO], f32)
                            dps = dpsp.tile([P, 2], f32)
                            last = len(active) - 1
                            for idx, zt in enumerate(active):
                                lhs = e_sb[:, zt, ms * P:(ms + 1) * P]
                                st = idx == 0
                                sp = idx == last
                                nc.tensor.matmul(ops[:, 0:NB], lhs,
                                                 vt_sb[:, zt, 0:NB],
                                                 start=st, stop=sp)
                                nc.tensor.matmul(ops[:, NB:DO], lhs,
                                                 vt_sb[:, zt, NB:DO],
                                                 start=st, stop=sp)
                                nc.tensor.matmul(dps, lhs, ones_sb,
                                                 start=st, stop=sp)
                            dr = drp.tile([P, 1], f32)
                            nc.vector.reciprocal(dr, dps[:, 0:1])
                            nc.vector.tensor_scalar_mul(ot, ops, dr)
                        else:
                            nc.vector.memset(ot, 0.0)
                        row = (i * 2 + ms) * P
                        nc.sync.dma_start(OTd[row:row + P, :], ot)

    nc.compile()
    return nc


def _prep_inputs(X, Z, mask, Wq, bq, Wk, bk, Wv, bv):
    f = np.float32
    X = np.ascontiguousarray(np.asarray(X, dtype=f))
    Z = np.ascontiguousarray(np.asarray(Z, dtype=f))
    mask = np.asarray(mask).astype(bool)
    Wq = np.asarray(Wq, dtype=f)
    Wk = np.asarray(Wk, dtype=f)
    Wv = np.asarray(Wv, dtype=f)
    bq = np.ascontiguousarray(np.asarray(bq, dtype=f)).reshape(DA, 1)
    bk = np.ascontiguousarray(np.asarray(bk, dtype=f)).reshape(DA, 1)
    bv = np.ascontiguousarray(np.asarray(bv, dtype=f)).reshape(DO, 1)

    status = _classify(mask)
    partial_pairs = [(zt, i) for i in range(NXB) for zt in range(NZT)
                     if status[zt, i] == 2]
    n_partial = max(1, len(partial_pairs))
    mbp = np.zeros((n_partial, P, BX), dtype=f)
    for j, (zt, i) in enumerate(partial_pairs):
        sub = mask[zt * P:(zt + 1) * P, i * BX:(i + 1) * BX]
        mbp[j] = np.where(sub, 0.0, NEG)

    common = {
        "MBP": mbp,
        "WqT": np.ascontiguousarray(Wq.T),
        "WkT": np.ascontiguousarray(Wk.T),
        "WvT": np.ascontiguousarray(Wv.T),
        "bq": bq,
        "bk": bk,
        "ones": np.ones((P, 2), dtype=f),
    }
    in_maps = [dict(common, X=np.ascontiguousarray(X[b]),
                    Z=np.ascontiguousarray(Z[b])) for b in range(BS)]
    return status, in_maps, bv


def kernel(X, Z, mask, Wq, bq, Wk, bk, Wv, bv):
    _, _, _, _, bass_utils = _get_concourse()
    status, in_maps, bv = _prep_inputs(X, Z, mask, Wq, bq, Wk, bk, Wv, bv)

    key = tuple(map(tuple, status))
    nc = _CACHE.get(key)
    if nc is None:
        nc = _build(key)
        _CACHE[key] = nc

    trace = os.environ.get("KERNEL_TRACE", "") == "1"
    res = bass_utils.run_bass_kernel_spmd(
        nc, in_maps, core_ids=list(range(BS)), trace=trace)
    if trace and res.exec_time_ns is not None:
        print(f"HW exec time: {res.exec_time_ns} ns")
        if res.instructions_and_trace is not None:
            print("trace:", res.instructions_and_trace[1])

    out = np.empty((BS, DO, LX), dtype=np.float32)
    for b in range(BS):
        out[b] = res.results[b]["OT"].T
    out += bv[None, :, :]
    return out


# revision 8
# speedup vs baseline: 1.0231x; 1.0231x over previous
"""Self-contained Trainium2 Bass kernel for single-head attention.

Problem (per batch b of 8):
    q = Wq @ X[b] + bq            (dattn=1024, lx=2048)
    k = Wk @ Z[b] + bk            (dattn=1024, lz=2048)
    v = Wv @ Z[b] + bv            (dout=1024,  lz=2048)
    S = k^T q                     (lz, lx)
    attn = softmax(where(mask, S, -inf) / sqrt(dattn), axis=lz)
    out[b] = v @ attn             (dout, lx)

Strategy:
  * Pure data parallelism: core b computes batch b (8 batches / 8 cores, no
    collectives).
  * All matmuls run as float32r (TF32-like, ~13 mantissa bits) which is 4x
    faster than fp32 on the PE array for moving dims >= 256.  Every SBUF
    tile feeding a matmul is declared float32r and produced as such
    (DMA bitcast or engine output conversion) to satisfy walrus.
  * Softmax without max-subtraction (scores are O(1) after the 1/32 scale, so
    exp never overflows): E = exp((S + maskbias)/32) is produced directly in
    (z, x) layout.  The denominator D[x] = sum_z E[z,x] is computed with a
    ones vector as the *stationary* matmul operand (out partitions = 2), and
    the output is built transposed and unnormalized: OT = E^T @ vT.  OT and D
    are shipped to the host, which divides, transposes, and adds bv (exact:
    attention columns sum to 1, so the bv contribution is bv broadcast).
  * The boolean mask is classified on the host per (128-z-tile x 256-x-block)
    into skip / fully-unmasked / partial.  Skipped blocks generate no compute;
    partial blocks add a packed additive-bias tile (0 or -1e30).  This is
    fully general in the mask, and skips ~44% of attention work for the
    causal mask.
  * DMAs of weights and input chunks are split per 128-partition k-tile so
    the first matmul of each phase waits on ~0.5MB, not 4MB; input and PSUM
    pools are shared across the three projection phases so phase boundaries
    double-buffer instead of draining.
"""

import math
import os
import sys

import numpy as np

P = 128            # partitions
D = 1024           # dx = dz (contraction dim of the projections)
DA = 1024          # dattn
DO = 1024          # dout
LX = 2048
LZ = 2048
BS = 8
KT = D // P        # contraction tiles for projections (8)
MA = DA // P       # dattn tiles (8)
NZT = LZ // P      # z tiles (16)
BX = 256           # attention x-block
NXB = LX // BX     # 8
CH = 512           # projection-phase column chunk
NB = 512           # PSUM bank free-dim (fp32)
SCALE = 1.0 / math.sqrt(DA)
NEG = -1.0e30

_CACHE = {}


def _get_concourse():
    try:
        import concourse.bass  # noqa: F401
    except ImportError:
        for p in ("/opt/trn_rl_repo", "/root/.axon_site/_ro/trn_rl_repo"):
            if os.path.isdir(p) and p not in sys.path:
                sys.path.insert(0, p)
    import concourse.bass as bass
    import concourse.mybir as mybir
    import concourse.tile as tile
    from concourse import bacc, bass_utils

    return bass, mybir, tile, bacc, bass_utils


def _classify(mask):
    """Per (z-tile, x-block) mask status: 0 skip, 1 fully-unmasked, 2 partial."""
    status = np.zeros((NZT, NXB), dtype=np.int32)
    for zt in range(NZT):
        for i in range(NXB):
            sub = mask[zt * P:(zt + 1) * P, i * BX:(i + 1) * BX]
            if sub.all():
                status[zt, i] = 1
            elif sub.any():
                status[zt, i] = 2
    return status


def _build(status_key):
    bass, mybir, tile, bacc, bass_utils = _get_concourse()
    f32 = mybir.dt.float32
    f32r = mybir.dt.float32r
    AF = mybir.ActivationFunctionType
    ADD = mybir.AluOpType.add

    def r(ap):
        return ap.bitcast(f32r)

    status = np.array(status_key, dtype=np.int32).reshape(NZT, NXB)
    partial_pairs = [(zt, i) for i in range(NXB) for zt in range(NZT)
                     if status[zt, i] == 2]
    n_partial = max(1, len(partial_pairs))
    partial_idx = {pair: j for j, pair in enumerate(partial_pairs)}

    nc = bacc.Bacc("TRN2", target_bir_lowering=False, debug=False,
                   num_devices=1)
    Xd = nc.dram_tensor("X", (D, LX), f32, kind="ExternalInput").ap()
    Zd = nc.dram_tensor("Z", (D, LZ), f32, kind="ExternalInput").ap()
    MBd = nc.dram_tensor("MBP", (n_partial, P, BX), f32,
                         kind="ExternalInput").ap()
    WqTd = nc.dram_tensor("WqT", (D, DA), f32, kind="ExternalInput").ap()
    WkTd = nc.dram_tensor("WkT", (D, DA), f32, kind="ExternalInput").ap()
    WvTd = nc.dram_tensor("WvT", (D, DO), f32, kind="ExternalInput").ap()
    bqd = nc.dram_tensor("bq", (DA, 1), f32, kind="ExternalInput").ap()
    bkd = nc.dram_tensor("bk", (DA, 1), f32, kind="ExternalInput").ap()
    onesd = nc.dram_tensor("ones", (P, 2), f32, kind="ExternalInput").ap()
    qsd = nc.dram_tensor("qs", (DA, LX), f32, kind="Internal").ap()
    OTd = nc.dram_tensor("OT", (LX, DO), f32, kind="ExternalOutput").ap()
    Dd = nc.dram_tensor("Dn", (NXB, BX), f32, kind="ExternalOutput").ap()

    xv = r(Xd.rearrange("(t p) l -> p t l", p=P))
    zv = r(Zd.rearrange("(t p) l -> p t l", p=P))
    wqv = r(WqTd.rearrange("(t p) d -> p t d", p=P))
    wkv = r(WkTd.rearrange("(t p) d -> p t d", p=P))
    wvv = r(WvTd.rearrange("(t p) d -> p t d", p=P))
    qsv = r(qsd.rearrange("(t p) l -> p t l", p=P))

    def dma_split(dst3, src3, n=KT, piece=256):
        """Per-(k-tile x column-piece) DMA: ~128KB pieces spread across the
        16 HWDGE queues so a phase's first matmul waits on one piece, not a
        512KB single-queue transfer (~23us at ~22GB/s/queue)."""
        cols = dst3.shape[2]
        for kt in range(n):
            for c0 in range(0, cols, piece):
                c1 = min(c0 + piece, cols)
                nc.sync.dma_start(dst3[:, kt, c0:c1], src3[:, kt, c0:c1])

    with tile.TileContext(nc) as tc:
        with tc.tile_pool(name="const", bufs=1) as cpool, \
             tc.tile_pool(name="kres", bufs=1) as kpool, \
             tc.tile_pool(name="vres", bufs=1) as vpool:
            bq_sb = cpool.tile([P, MA, 1], f32)
            nc.sync.dma_start(bq_sb, bqd.rearrange("(t p) o -> p t o", p=P))
            bk_sb = cpool.tile([P, MA, 1], f32)
            nc.sync.dma_start(bk_sb, bkd.rearrange("(t p) o -> p t o", p=P))
            ones_sb = cpool.tile([P, 2], f32r)
            nc.sync.dma_start(ones_sb, r(onesd))

            k_sb = kpool.tile([P, MA, LZ], f32r)      # k: (dattn, lz)
            vt_sb = vpool.tile([P, NZT, DO], f32r)    # v^T: (lz, dout)

            # ---- Projection phases share the input + PSUM pools ----
            prj = tc.alloc_tile_pool(name="zin", bufs=2)
            zinp = prj
            psp = tc.alloc_tile_pool(name="psprj", bufs=4, space="PSUM")

            # ---- Phase V: vT = Z^T @ WvT ----
            with tc.tile_pool(name="wv", bufs=1) as wvp:
                wvt_sb = wvp.tile([P, KT, DO], f32r)
                dma_split(wvt_sb, wvv)
                for c in range(LZ // CH):
                    z_sb = zinp.tile([P, KT, CH], f32r, name="z_sb")
                    dma_split(z_sb, zv[:, :, c * CH:(c + 1) * CH])
                    for m in range(CH // P):
                        for n in range(DO // NB):
                            vps = psp.tile([P, NB], f32, name="prjps")
                            for kt in range(KT):
                                nc.tensor.matmul(
                                    vps,
                                    z_sb[:, kt, m * P:(m + 1) * P],
                                    wvt_sb[:, kt, n * NB:(n + 1) * NB],
                                    start=(kt == 0), stop=(kt == KT - 1))
                            nc.vector.tensor_copy(
                                vt_sb[:, c * (CH // P) + m,
                                      n * NB:(n + 1) * NB], vps)

            # ---- Phase K: k = Wk @ Z + bk ----
            with tc.tile_pool(name="wk", bufs=1) as wkp:
                wkt_sb = wkp.tile([P, KT, DA], f32r)
                dma_split(wkt_sb, wkv)
                for c in range(LZ // CH):
                    z_sb = zinp.tile([P, KT, CH], f32r, name="z_sb")
                    dma_split(z_sb, zv[:, :, c * CH:(c + 1) * CH])
                    for m in range(MA):
                        kps = psp.tile([P, CH], f32, name="prjps")
                        for kt in range(KT):
                            nc.tensor.matmul(
                                kps,
                                wkt_sb[:, kt, m * P:(m + 1) * P],
                                z_sb[:, kt, :],
                                start=(kt == 0), stop=(kt == KT - 1))
                        nc.scalar.activation(
                            k_sb[:, m, c * CH:(c + 1) * CH], kps,
                            AF.Identity, bias=bk_sb[:, m, :], scale=1.0)

            # ---- Phase Q: q = Wq @ X + bq, spilled to DRAM ----
            with tc.tile_pool(name="wq", bufs=1) as wqp, \
                 tc.tile_pool(name="qst", bufs=4) as qstp:
                wqt_sb = wqp.tile([P, KT, DA], f32r)
                dma_split(wqt_sb, wqv)
                for c in range(LX // CH):
                    z_sb = zinp.tile([P, KT, CH], f32r, name="z_sb")
                    dma_split(z_sb, xv[:, :, c * CH:(c + 1) * CH])
                    for m in range(MA):
                        qps = psp.tile([P, CH], f32, name="prjps")
                        for kt in range(KT):
                            nc.tensor.matmul(
                                qps,
                                wqt_sb[:, kt, m * P:(m + 1) * P],
                                z_sb[:, kt, :],
                                start=(kt == 0), stop=(kt == KT - 1))
                        qstage = qstp.tile([P, CH], f32r)
                        nc.scalar.activation(qstage, qps, AF.Identity,
                                             bias=bq_sb[:, m, :], scale=1.0)
                        nc.sync.dma_start(
                            qsv[:, m, c * CH:(c + 1) * CH], qstage)

            zinp.release()
            psp.release()

            # ---- Attention ----
            with tc.tile_pool(name="qblk", bufs=2) as qblkp, \
                 tc.tile_pool(name="ebuf", bufs=2) as epool, \
                 tc.tile_pool(name="mbuf", bufs=4) as mpool, \
                 tc.tile_pool(name="otb", bufs=3) as otp, \
                 tc.tile_pool(name="dsb", bufs=2) as dsbp, \
                 tc.tile_pool(name="pss", bufs=2, space="PSUM") as spsp, \
                 tc.tile_pool(name="pso", bufs=2, space="PSUM") as opsp, \
                 tc.tile_pool(name="psd", bufs=2, space="PSUM") as dpsp:
                for i in range(NXB):
                    active = [zt for zt in range(NZT) if status[zt, i] != 0]
                    q_sb = qblkp.tile([P, MA, BX], f32r)
                    dma_split(q_sb, qsv[:, :, i * BX:(i + 1) * BX], n=MA)
                    e_sb = epool.tile([P, NZT, BX], f32r)
                    for zt in active:
                        sps = spsp.tile([P, BX], f32)
                        for kt in range(MA):
                            nc.tensor.matmul(
                                sps,
                                k_sb[:, kt, zt * P:(zt + 1) * P],
                                q_sb[:, kt, :],
                                start=(kt == 0), stop=(kt == MA - 1))
                        if status[zt, i] == 2:
                            mb_sb = mpool.tile([P, BX], f32)
                            nc.sync.dma_start(
                                mb_sb, MBd[partial_idx[(zt, i)]])
                            nc.vector.tensor_tensor(sps, sps, mb_sb, op=ADD)
                        nc.scalar.activation(e_sb[:, zt, :], sps, AF.Exp,
                                             scale=SCALE)
                    if active:
                        # D[x] = sum_z E[z, x]: ones as stationary operand
                        dps = dpsp.tile([2, BX], f32)
                        last = len(active) - 1
                        for idx, zt in enumerate(active):
                            nc.tensor.matmul(dps, ones_sb, e_sb[:, zt, :],
                                             start=(idx == 0),
                                             stop=(idx == last))
                        d_sb = dsbp.tile([1, BX], f32)
                        nc.vector.tensor_copy(d_sb, dps[0:1, :])
                        nc.sync.dma_start(Dd[i:i + 1, :], d_sb)
                    for ms in range(BX // P):
                        ot = otp.tile([P, DO], f32)
                        if active:
                            ops = opsp.tile([P, DO], f32)
                            last = len(active) - 1
                            for idx, zt in enumerate(active):
                                lhs = e_sb[:, zt, ms * P:(ms + 1) * P]
                                st = idx == 0
                                sp = idx == last
                                nc.tensor.matmul(ops[:, 0:NB], lhs,
                                                 vt_sb[:, zt, 0:NB],
                                                 start=st, stop=sp)
                                nc.tensor.matmul(ops[:, NB:DO], lhs,
                                                 vt_sb[:, zt, NB:DO],
                                                 start=st, stop=sp)
                            nc.scalar.copy(ot, ops)
                        else:
                            nc.vector.memset(ot, 0.0)
                        row = (i * 2 + ms) * P
                        nc.sync.dma_start(OTd[row:row + P, :], ot)

    nc.compile()
    return nc


def _prep_inputs(X, Z, mask, Wq, bq, Wk, bk, Wv, bv):
    f = np.float32
    X = np.ascontiguousarray(np.asarray(X, dtype=f))
    Z = np.ascontiguousarray(np.asarray(Z, dtype=f))
    mask = np.asarray(mask).astype(bool)
    Wq = np.asarray(Wq, dtype=f)
    Wk = np.asarray(Wk, dtype=f)
    Wv = np.asarray(Wv, dtype=f)
    bq = np.ascontiguousarray(np.asarray(bq, dtype=f)).reshape(DA, 1)
    bk = np.ascontiguousarray(np.asarray(bk, dtype=f)).reshape(DA, 1)
    bv = np.ascontiguousarray(np.asarray(bv, dtype=f)).reshape(DO, 1)

    status = _classify(mask)
    partial_pairs = [(zt, i) for i in range(NXB) for zt in range(NZT)
                     if status[zt, i] == 2]
    n_partial = max(1, len(partial_pairs))
    mbp = np.zeros((n_partial, P, BX), dtype=f)
    for j, (zt, i) in enumerate(partial_pairs):
        sub = mask[zt * P:(zt + 1) * P, i * BX:(i + 1) * BX]
        mbp[j] = np.where(sub, 0.0, NEG)

    common = {
        "MBP": mbp,
        "WqT": np.ascontiguousarray(Wq.T),
        "WkT": np.ascontiguousarray(Wk.T),
        "WvT": np.ascontiguousarray(Wv.T),
        "bq": bq,
        "bk": bk,
        "ones": np.ones((P, 2), dtype=f),
    }
    in_maps = [dict(common, X=np.ascontiguousarray(X[b]),
                    Z=np.ascontiguousarray(Z[b])) for b in range(BS)]
    return status, in_maps, bv


def kernel(X, Z, mask, Wq, bq, Wk, bk, Wv, bv):
    _, _, _, _, bass_utils = _get_concourse()
    status, in_maps, bv = _prep_inputs(X, Z, mask, Wq, bq, Wk, bk, Wv, bv)

    key = tuple(map(tuple, status))
    nc = _CACHE.get(key)
    if nc is None:
        nc = _build(key)
        _CACHE[key] = nc

    trace = os.environ.get("KERNEL_TRACE", "") == "1"
    res = bass_utils.run_bass_kernel_spmd(
        nc, in_maps, core_ids=list(range(BS)), trace=trace)
    if trace and res.exec_time_ns is not None:
        print(f"HW exec time: {res.exec_time_ns} ns")
        if res.instructions_and_trace is not None:
            print("trace:", res.instructions_and_trace[1])

    out = np.empty((BS, DO, LX), dtype=np.float32)
    for b in range(BS):
        ot = res.results[b]["OT"]                    # (LX, DO) unnormalized
        dn = res.results[b]["Dn"].reshape(LX)        # softmax denominators
        dn = np.where(dn == 0.0, 1.0, dn)
        out[b] = (ot / dn[:, None]).T
    out += bv[None, :, :]
    return out


# revision 9
# speedup vs baseline: 1.1205x; 1.0952x over previous
"""Self-contained Trainium2 Bass kernel for single-head attention.

Problem (per batch b of 8):
    q = Wq @ X[b] + bq            (dattn=1024, lx=2048)
    k = Wk @ Z[b] + bk            (dattn=1024, lz=2048)
    v = Wv @ Z[b] + bv            (dout=1024,  lz=2048)
    S = k^T q                     (lz, lx)
    attn = softmax(where(mask, S, -inf) / sqrt(dattn), axis=lz)
    out[b] = v @ attn             (dout, lx)

Strategy:
  * Pure data parallelism: core b computes batch b (8 batches / 8 cores, no
    collectives).
  * All matmuls run as float32r (TF32-like, ~13 mantissa bits) which is 4x
    faster than fp32 on the PE array for moving dims >= 256.  Every SBUF
    tile feeding a matmul is declared float32r and produced as such
    (DMA bitcast or engine output conversion) to satisfy walrus.
  * Softmax without max-subtraction (scores are O(1) after the 1/32 scale, so
    exp never overflows): E = exp((S + maskbias)/32) is produced directly in
    (z, x) layout.  The denominator D[x] = sum_z E[z,x] is computed with a
    ones vector as the *stationary* matmul operand (out partitions = 2), and
    the output is built transposed and unnormalized: OT = E^T @ vT.  OT and D
    are shipped to the host, which divides, transposes, and adds bv (exact:
    attention columns sum to 1, so the bv contribution is bv broadcast).
  * The boolean mask is classified on the host per (128-z-tile x 256-x-block)
    into skip / fully-unmasked / partial.  Skipped blocks generate no compute;
    partial blocks add a packed additive-bias tile (0 or -1e30).  This is
    fully general in the mask, and skips ~44% of attention work for the
    causal mask.
  * DMAs of weights and input chunks are split per 128-partition k-tile so
    the first matmul of each phase waits on ~0.5MB, not 4MB; input and PSUM
    pools are shared across the three projection phases so phase boundaries
    double-buffer instead of draining.
"""

import math
import os
import sys

import numpy as np

P = 128            # partitions
D = 1024           # dx = dz (contraction dim of the projections)
DA = 1024          # dattn
DO = 1024          # dout
LX = 2048
LZ = 2048
BS = 8
KT = D // P        # contraction tiles for projections (8)
MA = DA // P       # dattn tiles (8)
NZT = LZ // P      # z tiles (16)
BX = 256           # attention x-block
NXB = LX // BX     # 8
CH = 512           # projection-phase column chunk
NB = 512           # PSUM bank free-dim (fp32)
SCALE = 1.0 / math.sqrt(DA)
NEG = -1.0e30

_CACHE = {}


def _get_concourse():
    try:
        import concourse.bass  # noqa: F401
    except ImportError:
        for p in ("/opt/trn_rl_repo", "/root/.axon_site/_ro/trn_rl_repo"):
            if os.path.isdir(p) and p not in sys.path:
                sys.path.insert(0, p)
    import concourse.bass as bass
    import concourse.mybir as mybir
    import concourse.tile as tile
    from concourse import bacc, bass_utils

    return bass, mybir, tile, bacc, bass_utils


def _classify(mask):
    """Per (z-tile, x-block) mask status: 0 skip, 1 fully-unmasked, 2 partial."""
    status = np.zeros((NZT, NXB), dtype=np.int32)
    for zt in range(NZT):
        for i in range(NXB):
            sub = mask[zt * P:(zt + 1) * P, i * BX:(i + 1) * BX]
            if sub.all():
                status[zt, i] = 1
            elif sub.any():
                status[zt, i] = 2
    return status


def _build(status_key):
    bass, mybir, tile, bacc, bass_utils = _get_concourse()
    f32 = mybir.dt.float32
    f32r = mybir.dt.float32r
    AF = mybir.ActivationFunctionType
    ADD = mybir.AluOpType.add

    def r(ap):
        return ap.bitcast(f32r)

    status = np.array(status_key, dtype=np.int32).reshape(NZT, NXB)
    partial_pairs = [(zt, i) for i in range(NXB) for zt in range(NZT)
                     if status[zt, i] == 2]
    n_partial = max(1, len(partial_pairs))
    partial_idx = {pair: j for j, pair in enumerate(partial_pairs)}

    nc = bacc.Bacc("TRN2", target_bir_lowering=False, debug=False,
                   num_devices=1)
    Xd = nc.dram_tensor("X", (D, LX), f32, kind="ExternalInput").ap()
    Zd = nc.dram_tensor("Z", (D, LZ), f32, kind="ExternalInput").ap()
    MBd = nc.dram_tensor("MBP", (n_partial, P, BX), f32,
                         kind="ExternalInput").ap()
    WqTd = nc.dram_tensor("WqT", (D, DA), f32, kind="ExternalInput").ap()
    WkTd = nc.dram_tensor("WkT", (D, DA), f32, kind="ExternalInput").ap()
    WvTd = nc.dram_tensor("WvT", (D, DO), f32, kind="ExternalInput").ap()
    bqd = nc.dram_tensor("bq", (DA, 1), f32, kind="ExternalInput").ap()
    bkd = nc.dram_tensor("bk", (DA, 1), f32, kind="ExternalInput").ap()
    onesd = nc.dram_tensor("ones", (P, 2), f32, kind="ExternalInput").ap()
    qsd = nc.dram_tensor("qs", (DA, LX), f32, kind="Internal").ap()
    OTd = nc.dram_tensor("OT", (LX, DO), f32, kind="ExternalOutput").ap()
    Dd = nc.dram_tensor("Dn", (NXB, BX), f32, kind="ExternalOutput").ap()

    xv = r(Xd.rearrange("(t p) l -> p t l", p=P))
    zv = r(Zd.rearrange("(t p) l -> p t l", p=P))
    wqv = r(WqTd.rearrange("(t p) d -> p t d", p=P))
    wkv = r(WkTd.rearrange("(t p) d -> p t d", p=P))
    wvv = r(WvTd.rearrange("(t p) d -> p t d", p=P))
    qsv = r(qsd.rearrange("(t p) l -> p t l", p=P))

    def dma_split(dst3, src3, n=KT):
        """Per-k-tile DMA so downstream matmuls wait on 1 slice, not all."""
        for kt in range(n):
            nc.sync.dma_start(dst3[:, kt], src3[:, kt])

    def dma_split_w(dst3, src3, n=KT, piece=NB):
        """Weight DMA, column-major pieces: the first PSUM group of a phase
        only needs columns [0:512) of every k-tile, so emit those first."""
        cols = dst3.shape[2]
        for c0 in range(0, cols, piece):
            c1 = min(c0 + piece, cols)
            for kt in range(n):
                nc.sync.dma_start(dst3[:, kt, c0:c1], src3[:, kt, c0:c1])

    with tile.TileContext(nc) as tc:
        with tc.tile_pool(name="const", bufs=1) as cpool, \
             tc.tile_pool(name="kres", bufs=1) as kpool, \
             tc.tile_pool(name="vres", bufs=1) as vpool:
            bq_sb = cpool.tile([P, MA, 1], f32)
            nc.sync.dma_start(bq_sb, bqd.rearrange("(t p) o -> p t o", p=P))
            bk_sb = cpool.tile([P, MA, 1], f32)
            nc.sync.dma_start(bk_sb, bkd.rearrange("(t p) o -> p t o", p=P))
            ones_sb = cpool.tile([P, 2], f32r)
            nc.sync.dma_start(ones_sb, r(onesd))

            k_sb = kpool.tile([P, MA, LZ], f32r)      # k: (dattn, lz)
            vt_sb = vpool.tile([P, NZT, DO], f32r)    # v^T: (lz, dout)

            # ---- Projection phases share the input + PSUM pools ----
            prj = tc.alloc_tile_pool(name="zin", bufs=2)
            zinp = prj
            psp = tc.alloc_tile_pool(name="psprj", bufs=4, space="PSUM")

            # ---- Phase V: vT = Z^T @ WvT ----
            with tc.tile_pool(name="wv", bufs=1) as wvp:
                wvt_sb = wvp.tile([P, KT, DO], f32r)
                dma_split_w(wvt_sb, wvv)
                vchunks = [(0, 256), (256, 256)] + [
                    (o, CH) for o in range(CH, LZ, CH)]
                for z0, zw in vchunks:
                    z_sb = zinp.tile([P, KT, CH], f32r, name="z_sb")
                    dma_split(z_sb[:, :, 0:zw], zv[:, :, z0:z0 + zw])
                    for m in range(zw // P):
                        for n in range(DO // NB):
                            vps = psp.tile([P, NB], f32, name="prjps")
                            for kt in range(KT):
                                nc.tensor.matmul(
                                    vps,
                                    z_sb[:, kt, m * P:(m + 1) * P],
                                    wvt_sb[:, kt, n * NB:(n + 1) * NB],
                                    start=(kt == 0), stop=(kt == KT - 1))
                            nc.vector.tensor_copy(
                                vt_sb[:, (z0 // P) + m,
                                      n * NB:(n + 1) * NB], vps)

            # ---- Phase K: k = Wk @ Z + bk ----
            with tc.tile_pool(name="wk", bufs=1) as wkp:
                wkt_sb = wkp.tile([P, KT, DA], f32r)
                dma_split_w(wkt_sb, wkv)
                for c in range(LZ // CH):
                    z_sb = zinp.tile([P, KT, CH], f32r, name="z_sb")
                    dma_split(z_sb, zv[:, :, c * CH:(c + 1) * CH])
                    for m in range(MA):
                        kps = psp.tile([P, CH], f32, name="prjps")
                        for kt in range(KT):
                            nc.tensor.matmul(
                                kps,
                                wkt_sb[:, kt, m * P:(m + 1) * P],
                                z_sb[:, kt, :],
                                start=(kt == 0), stop=(kt == KT - 1))
                        nc.scalar.activation(
                            k_sb[:, m, c * CH:(c + 1) * CH], kps,
                            AF.Identity, bias=bk_sb[:, m, :], scale=1.0)

            # ---- Phase Q: q = Wq @ X + bq, spilled to DRAM ----
            with tc.tile_pool(name="wq", bufs=1) as wqp, \
                 tc.tile_pool(name="qst", bufs=4) as qstp:
                wqt_sb = wqp.tile([P, KT, DA], f32r)
                dma_split_w(wqt_sb, wqv)
                for c in range(LX // CH):
                    z_sb = zinp.tile([P, KT, CH], f32r, name="z_sb")
                    dma_split(z_sb, xv[:, :, c * CH:(c + 1) * CH])
                    for m in range(MA):
                        qps = psp.tile([P, CH], f32, name="prjps")
                        for kt in range(KT):
                            nc.tensor.matmul(
                                qps,
                                wqt_sb[:, kt, m * P:(m + 1) * P],
                                z_sb[:, kt, :],
                                start=(kt == 0), stop=(kt == KT - 1))
                        qstage = qstp.tile([P, CH], f32r)
                        nc.scalar.activation(qstage, qps, AF.Identity,
                                             bias=bq_sb[:, m, :], scale=1.0)
                        nc.sync.dma_start(
                            qsv[:, m, c * CH:(c + 1) * CH], qstage)

            zinp.release()
            psp.release()

            # ---- Attention ----
            with tc.tile_pool(name="qblk", bufs=2) as qblkp, \
                 tc.tile_pool(name="ebuf", bufs=2) as epool, \
                 tc.tile_pool(name="mbuf", bufs=4) as mpool, \
                 tc.tile_pool(name="otb", bufs=3) as otp, \
                 tc.tile_pool(name="dsb", bufs=2) as dsbp, \
                 tc.tile_pool(name="pss", bufs=2, space="PSUM") as spsp, \
                 tc.tile_pool(name="pso", bufs=2, space="PSUM") as opsp, \
                 tc.tile_pool(name="psd", bufs=2, space="PSUM") as dpsp:
                for i in range(NXB):
                    active = [zt for zt in range(NZT) if status[zt, i] != 0]
                    q_sb = qblkp.tile([P, MA, BX], f32r)
                    dma_split(q_sb, qsv[:, :, i * BX:(i + 1) * BX], n=MA)
                    e_sb = epool.tile([P, NZT, BX], f32r)
                    for zt in active:
                        sps = spsp.tile([P, BX], f32)
                        for kt in range(MA):
                            nc.tensor.matmul(
                                sps,
                                k_sb[:, kt, zt * P:(zt + 1) * P],
                                q_sb[:, kt, :],
                                start=(kt == 0), stop=(kt == MA - 1))
                        if status[zt, i] == 2:
                            mb_sb = mpool.tile([P, BX], f32)
                            nc.sync.dma_start(
                                mb_sb, MBd[partial_idx[(zt, i)]])
                            nc.vector.tensor_tensor(sps, sps, mb_sb, op=ADD)
                        nc.scalar.activation(e_sb[:, zt, :], sps, AF.Exp,
                                             scale=SCALE)
                    if active:
                        # D[x] = sum_z E[z, x]: ones as stationary operand
                        dps = dpsp.tile([2, BX], f32)
                        last = len(active) - 1
                        for idx, zt in enumerate(active):
                            nc.tensor.matmul(dps, ones_sb, e_sb[:, zt, :],
                                             start=(idx == 0),
                                             stop=(idx == last))
                        d_sb = dsbp.tile([1, BX], f32)
                        nc.vector.tensor_copy(d_sb, dps[0:1, :])
                        nc.sync.dma_start(Dd[i:i + 1, :], d_sb)
                    for ms in range(BX // P):
                        ot = otp.tile([P, DO], f32)
                        if active:
                            ops = opsp.tile([P, DO], f32)
                            last = len(active) - 1
                            for idx, zt in enumerate(active):
                                lhs = e_sb[:, zt, ms * P:(ms + 1) * P]
                                st = idx == 0
                                sp = idx == last
                                nc.tensor.matmul(ops[:, 0:NB], lhs,
                                                 vt_sb[:, zt, 0:NB],
                                                 start=st, stop=sp)
                                nc.tensor.matmul(ops[:, NB:DO], lhs,
                                                 vt_sb[:, zt, NB:DO],
                                                 start=st, stop=sp)
                            nc.scalar.copy(ot, ops)
                        else:
                            nc.vector.memset(ot, 0.0)
                        row = (i * 2 + ms) * P
                        nc.sync.dma_start(OTd[row:row + P, :], ot)

    nc.compile()
    return nc


def _prep_inputs(X, Z, mask, Wq, bq, Wk, bk, Wv, bv):
    f = np.float32
    X = np.ascontiguousarray(np.asarray(X, dtype=f))
    Z = np.ascontiguousarray(np.asarray(Z, dtype=f))
    mask = np.asarray(mask).astype(bool)
    Wq = np.asarray(Wq, dtype=f)
    Wk = np.asarray(Wk, dtype=f)
    Wv = np.asarray(Wv, dtype=f)
    bq = np.ascontiguousarray(np.asarray(bq, dtype=f)).reshape(DA, 1)
    bk = np.ascontiguousarray(np.asarray(bk, dtype=f)).reshape(DA, 1)
    bv = np.ascontiguousarray(np.asarray(bv, dtype=f)).reshape(DO, 1)

    status = _classify(mask)
    partial_pairs = [(zt, i) for i in range(NXB) for zt in range(NZT)
                     if status[zt, i] == 2]
    n_partial = max(1, len(partial_pairs))
    mbp = np.zeros((n_partial, P, BX), dtype=f)
    for j, (zt, i) in enumerate(partial_pairs):
        sub = mask[zt * P:(zt + 1) * P, i * BX:(i + 1) * BX]
        mbp[j] = np.where(sub, 0.0, NEG)

    common = {
        "MBP": mbp,
        "WqT": np.ascontiguousarray(Wq.T),
        "WkT": np.ascontiguousarray(Wk.T),
        "WvT": np.ascontiguousarray(Wv.T),
        "bq": bq,
        "bk": bk,
        "ones": np.ones((P, 2), dtype=f),
    }
    in_maps = [dict(common, X=np.ascontiguousarray(X[b]),
                    Z=np.ascontiguousarray(Z[b])) for b in range(BS)]
    return status, in_maps, bv


def kernel(X, Z, mask, Wq, bq, Wk, bk, Wv, bv):
    _, _, _, _, bass_utils = _get_concourse()
    status, in_maps, bv = _prep_inputs(X, Z, mask, Wq, bq, Wk, bk, Wv, bv)

    key = tuple(map(tuple, status))
    nc = _CACHE.get(key)
    if nc is None:
        nc = _build(key)
        _CACHE[key] = nc

    trace = os.environ.get("KERNEL_TRACE", "") == "1"
    res = bass_utils.run_bass_kernel_spmd(
        nc, in_maps, core_ids=list(range(BS)), trace=trace)
    if trace and res.exec_time_ns is not None:
        print(f"HW exec time: {res.exec_time_ns} ns")
        if res.instructions_and_trace is not None:
            print("trace:", res.instructions_and_trace[1])

    out = np.empty((BS, DO, LX), dtype=np.float32)
    for b in range(BS):
        ot = res.results[b]["OT"]                    # (LX, DO) unnormalized
        dn = res.results[b]["Dn"].reshape(LX)        # softmax denominators
        dn = np.where(dn == 0.0, 1.0, dn)
        out[b] = (ot / dn[:, None]).T
    out += bv[None, :, :]
    return out
